# revision 31
# baseline (speedup 1.0000x reference)
"""Decoupled top-k distillation loss on 8 Trainium2 NeuronCores.

Full inputs: student_logits, teacher_logits (2, 2048, 32000) f32.
Data-parallel: 4096 flattened rows sharded 512/core across 8 cores.

v5: host fp16-pair packing of x/16 (halves HBM traffic) + balanced
ACT/DVE exp split with a single fused custom DVE op + host top-32.

  - Host packs each (teacher, student) element pair into one f32:
    high 16 bits = fp16(t/16), low 16 bits = fp16(s/16). Dividing by
    2^4 is exact in fp16 and preserves ordering, so the f32 view still
    orders like t and DVE max8 selects top (t, s) PAIRS; the host
    multiplies the unpacked support values back by 16.
  - DVE max8 per 2000-block -> 128 candidate pairs/tile, shipped to
    the host, which takes the top-32 (no on-device cascade).
  - exp sums S_t/S_s are split by column range to balance engines:
      ACT: strided fp16 exp passes, scale=16 (free), accum_out (exact).
      DVE: fused custom op on cols [CA, 8000) (7 ALU stages + accum):
        EXP16_SUM_ANT: accum += ((x + c1)*x + c2)^16
      with (c1, c2) fitted so ((x+c1)x+c2)^16 ~ c2^16 * e^(16x) with
      ~1e-6 exp-weighted bias for x*16 ~ N(0,1); host divides partials
      by c2^16. All arithmetic stays in the DVE's f32 pipeline.
  - Host computes BCE + truncated KL in f64 exactly as the reference
    does on the selected support.
"""

import sys

import numpy as np

sys.path.insert(0, "/opt/trn_rl_repo")

import concourse.bacc as bacc  # noqa: E402
import concourse.bass as bass  # noqa: E402,F401
import concourse.dve_ops as dops  # noqa: E402
import concourse.mybir as mybir  # noqa: E402
from concourse.bass_utils import run_bass_kernel_spmd  # noqa: E402
from concourse.dve_spec import (  # noqa: E402
    C0, C1, AluOp, Bin, Spec, Src0, lower,
)
from concourse.dve_uop import DveOpSpec  # noqa: E402
from concourse.tile import TileContext  # noqa: E402

F32 = mybir.dt.float32
FP16 = mybir.dt.float16
BF16 = mybir.dt.bfloat16
ALU = mybir.AluOpType
ACTF = mybir.ActivationFunctionType
AX = mybir.AxisListType

B, L, V = 2, 2048, 32000
N = B * L                  # 4096 rows
NCORES = 8
ROWS = N // NCORES         # 512 rows per core
P = 128                    # rows per tile (partition dim)
NT = ROWS // P             # 4 tiles per core
K = 32
SC = 8000                  # standard chunk width (packed f32 elements)
BLK = 4000                 # selection block width (top-8 per block)
NCAND = 64                 # 8 blocks * 8 candidates per tile
CA = 6600                  # columns handled by ACT exp per 8000-chunk
DW = SC - CA               # columns handled by DVE exp per 8000-chunk


def _plan(it):
    """Per-tile chunk plan: list of (start, width, act_width).

    Tile 0 leads with four 4000-col chunks so the first compute starts
    after a 2 MB DMA instead of 4 MB and the pipeline fills while the
    first full-size chunks stream in (shorter ramp). The very last
    chunk shifts ~1500 exp columns from ACT to DVE: the selection work
    (max8) all lands on DVE mid-kernel, so at the tail DVE would
    otherwise sit idle while ACT finishes."""
    ws = [4000] * 4 + [8000] * 2 if it == 0 else [8000] * 4
    out = []
    c0 = 0
    for i, w in enumerate(ws):
        ca = (w * CA) // 8000
        if it == NT - 1 and i == len(ws) - 1:
            ca = 5100
        out.append((c0, w, ca))
        c0 += w
    return out


_OCOLS_T = [4 * len(_plan(it)) for it in range(NT)]   # accum cols per tile
_OOFF = [sum(_OCOLS_T[:i]) for i in range(NT)]        # per-tile col offsets
XSCALE = 16.0              # host pre-divides logits by this (exact in fp16)
EC1 = 1.943617330549676    # fused-exp fit: q = (x + EC1)*x + EC2
EC2 = 1.9421392252956275
ESCALE = EC2 ** 16         # DVE partials are scaled by this


def _register_exp_ops():
    """Monkeypatch the fused exp custom DVE op into the concourse
    registry. Pure-python registration: the per-NEFF ucode table is
    generated from the Spec at compile time (dve_table_for_ops), so no
    repo files change. The sha is computed at runtime to satisfy the
    drift check."""
    if any(op.name == "EXP16_SUM_ANT" for op in dops.OPS):
        return

    q = (Src0 + C0) * Src0 + C1
    for _ in range(4):
        q = Bin(AluOp.MULTIPLY, q, q)

    def ref(in0, in1, c0, c1, c2):
        x = in0.astype(np.float32)
        qq = (x + np.float32(c0)) * x + np.float32(c1)
        for _ in range(4):
            qq = qq * qq
        return qq, qq.sum(axis=-1, keepdims=True).astype(np.float32)

    sp = Spec(body=q, accum=AluOp.ADD, reference=ref)
    nm = "EXP16_SUM_ANT"
    dops._SUB_OPCODE_FOR_NAME[nm] = max(dops._SUB_OPCODE_FOR_NAME.values()) + 1
    assert dops._SUB_OPCODE_FOR_NAME[nm] < 0x20
    sha = DveOpSpec(name=nm, opcode=dops.get_dve_sub_opcode(nm),
                    uops=lower(sp, ver="v3"), rd1_en=False).sha("v3")
    op = dops.DveOp(nm, sp, subdim=False, uops_sha={"v3": sha})
    dops.OPS.append(op)
    dops.CUSTOM_DVE_SPECS[nm] = sp


_register_exp_ops()
_EXP16 = next(op for op in dops.OPS if op.name == "EXP16_SUM_ANT")


def build_nc(nt=NT):
    rows = nt * P
    nc = bacc.Bacc("TRN2", debug=False)
    ocols = sum(_OCOLS_T)
    p_in = nc.declare_dram_parameter("p", [rows, V], F32, isOutput=False)
    o_out = nc.declare_dram_parameter("o", [P, ocols], F32, isOutput=True)
    c_out = nc.declare_dram_parameter("c", [P, NCAND * nt], F32, isOutput=True)

    with TileContext(nc) as tc:
        with (
            tc.tile_pool(name="pck", bufs=4) as pck,
            tc.tile_pool(name="cnd", bufs=2) as cnd,
            tc.tile_pool(name="singles", bufs=1) as singles,
        ):
            max_ca = max(ca for it in range(nt) for _, _, ca in _plan(it))
            max_dw = max(w - ca for it in range(nt) for _, w, ca in _plan(it))
            out_t = singles.tile([P, ocols], F32)
            dump_a = singles.tile([P, max_ca], FP16)    # ACT exp dump
            dump_v = singles.tile([P, max_dw], BF16)    # DVE exp16 dump

            for it in range(nt):
                r0 = it * P
                oc = _OOFF[it]  # accum columns base (4 per chunk)
                cand = cnd.tile([P, NCAND], F32, tag="cand")
                g = 0  # block counter within the tile

                for u, (c0, w, ca) in enumerate(_plan(it)):
                    up = pck.tile([P, w], F32, tag=f"p{w}",
                                  name=f"p{it}_{u}")
                    # tile 0's first chunks go out on the ACT HWDGE ring:
                    # the sync engine spends ~13 us on preamble before it
                    # can issue, while ACT's sequencer is free early.
                    q = nc.scalar if (it == 0 and u < 2) else nc.sync
                    q.dma_start(
                        out=up, in_=p_in[r0:r0 + P, c0:c0 + w])

                    # ACT: S_t / S_s partials over cols [0, ca): strided
                    # fp16 exp(16*x) with free accumulator outputs.
                    af = up[:, 0:ca].bitcast(FP16)
                    nc.scalar.activation(
                        out=dump_a[:, 0:ca], in_=af[:, 1::2], func=ACTF.Exp,
                        scale=XSCALE,
                        accum_out=out_t[:, oc:oc + 1],
                    )
                    nc.scalar.activation(
                        out=dump_a[:, 0:ca], in_=af[:, 0::2], func=ACTF.Exp,
                        scale=XSCALE,
                        accum_out=out_t[:, oc + 1:oc + 2],
                    )

                    # DVE: c2^16 * exp partials over cols [ca, w)
                    dv = up[:, ca:w].bitcast(FP16)
                    nc.vector._custom_dve(
                        _EXP16, out=dump_v[:, 0:w - ca], in0=dv[:, 1::2],
                        s0=EC1, s1=EC2,
                        accum_out=out_t[:, oc + 2:oc + 3],
                    )
                    nc.vector._custom_dve(
                        _EXP16, out=dump_v[:, 0:w - ca], in0=dv[:, 0::2],
                        s0=EC1, s1=EC2,
                        accum_out=out_t[:, oc + 3:oc + 4],
                    )
                    oc += 4

                    # per-block top-8 of packed pairs -> candidate tile
                    for b0 in range(0, w, BLK):
                        nc.vector.max(
                            out=cand[:, g * 8:(g + 1) * 8],
                            in_=up[:, b0:b0 + BLK],
                        )
                        g += 1

                assert g * 8 == NCAND
                nc.sync.dma_start(
                    out=c_out[:, it * NCAND:(it + 1) * NCAND], in_=cand)

            nc.sync.dma_start(out=o_out[:, :], in_=out_t[:, :])

    nc.finalize()
    return nc


_NC_CACHE = None


def _get_nc():
    global _NC_CACHE
    if _NC_CACHE is None:
        _NC_CACHE = build_nc()
    return _NC_CACHE


def pack_pairs(t2d, s2d):
    """(N, V) f32 teacher/student -> packed u32
    (fp16(t/16)<<16 | fp16(s/16)) viewed as f32."""
    th = (t2d / XSCALE).astype(np.float16).view(np.uint16).astype(np.uint32)
    sh = (s2d / XSCALE).astype(np.float16).view(np.uint16).astype(np.uint32)
    return ((th << 16) | sh).view(np.float32)


def run_device(t2d, s2d, trace=False, **kw):
    """t2d/s2d: (N, V) float32. Returns BassKernelResults."""
    nc = _get_nc()
    p2d = pack_pairs(t2d, s2d)
    in_maps = []
    for c in range(NCORES):
        sl = slice(c * ROWS, (c + 1) * ROWS)
        in_maps.append({"p": np.ascontiguousarray(p2d[sl])})
    return run_bass_kernel_spmd(nc, in_maps, list(range(NCORES)), trace=trace,
                                **kw)


def _gather(res):
    """Device results -> (cand, s_t, s_s)."""
    cand = np.empty((N, NCAND), dtype=np.float32)
    s_t = np.empty(N, dtype=np.float64)
    s_s = np.empty(N, dtype=np.float64)
    for c in range(NCORES):
        o = np.asarray(res.results[c]["o"])  # [P, OCOLS*NT] f32
        cd = np.asarray(res.results[c]["c"])  # [P, NCAND*NT] f32
        for it in range(NT):
            r = slice(c * ROWS + it * P, c * ROWS + (it + 1) * P)
            cand[r] = cd[:, it * NCAND:(it + 1) * NCAND]
            o64 = o[:, _OOFF[it]:_OOFF[it] + _OCOLS_T[it]].astype(np.float64)
            # per chunk: st_a | ss_a | st_d | ss_d
            s_t[r] = o64[:, 0::4].sum(1) + o64[:, 2::4].sum(1) / ESCALE
            s_s[r] = o64[:, 1::4].sum(1) + o64[:, 3::4].sum(1) / ESCALE
    return cand, s_t, s_s


def kernel(student_logits, teacher_logits):
    s2d = np.asarray(student_logits, dtype=np.float32).reshape(N, V)
    t2d = np.asarray(teacher_logits, dtype=np.float32).reshape(N, V)
    res = run_device(t2d, s2d)
    cand, s_t, s_s = _gather(res)

    # sanity net: a (rare, first-execution) corrupted run shows up as
    # non-finite or non-positive sums / out-of-range teacher values.
    tmax = (cand.view(np.uint32)[:, 0] >> 16).astype(np.uint16).view(
        np.float16).astype(np.float64)
    if (not np.isfinite(s_t).all() or not np.isfinite(s_s).all()
            or (s_t <= 0).any() or (s_s <= 0).any()
            or not np.isfinite(tmax).all()):
        res = run_device(t2d, s2d)
        cand, s_t, s_s = _gather(res)

    # host top-32 of the candidate pairs per row (same f32 ordering as
    # the device max8: packed high bits = fp16 teacher)
    top32 = -np.sort(-cand, axis=1)[:, :K]
    p32 = top32.view(np.uint32)

    # unpack fp16 halves (values are x/16) -> t32, s32 (f64)
    t32 = (p32 >> 16).astype(np.uint16).view(np.float16).astype(
        np.float64) * XSCALE
    s32 = (p32 & 0xFFFF).astype(np.uint16).view(np.float16).astype(
        np.float64) * XSCALE

    # host finals in f64, replicating the reference on this support
    a_t = np.exp(t32).sum(1)
    p_t = a_t / s_t
    p_s = np.exp(s32).sum(1) / s_s

    log_ps = np.maximum(np.log(p_s), -100.0)
    log_1mps = np.maximum(np.log1p(-p_s), -100.0)
    loss_b = np.mean(-(p_t * log_ps + (1.0 - p_t) * log_1mps))

    th = t32 / 2.0
    sh = s32 / 2.0
    log_p = th - (np.log(np.exp(th - th.max(1, keepdims=True)).sum(1))
                  + th.max(1)).reshape(-1, 1)
    log_q = sh - (np.log(np.exp(sh - sh.max(1, keepdims=True)).sum(1))
                  + sh.max(1)).reshape(-1, 1)
    p = np.exp(log_p)
    loss_t = (p * (log_p - log_q)).sum(1).mean()

    return np.float32(loss_b + p_t.mean() * 4.0 * loss_t)


# revision 32
# speedup vs baseline: 1.0579x; 1.0579x over previous
"""Decoupled top-k distillation loss on 8 Trainium2 NeuronCores.

Full inputs: student_logits, teacher_logits (2, 2048, 32000) f32.
Data-parallel: 4096 flattened rows sharded 512/core across 8 cores.

v5: host fp16-pair packing of x/16 (halves HBM traffic) + balanced
ACT/DVE exp split with a single fused custom DVE op + host top-32.

  - Host packs each (teacher, student) element pair into one f32:
    high 16 bits = fp16(t/16), low 16 bits = fp16(s/16). Dividing by
    2^4 is exact in fp16 and preserves ordering, so the f32 view still
    orders like t and DVE max8 selects top (t, s) PAIRS; the host
    multiplies the unpacked support values back by 16.
  - DVE max8 per 2000-block -> 128 candidate pairs/tile, shipped to
    the host, which takes the top-32 (no on-device cascade).
  - exp sums S_t/S_s are split by column range to balance engines:
      ACT: strided fp16 exp passes, scale=16 (free), accum_out (exact).
      DVE: fused custom op on cols [CA, 8000) (7 ALU stages + accum):
        EXP16_SUM_ANT: accum += ((x + c1)*x + c2)^16
      with (c1, c2) fitted so ((x+c1)x+c2)^16 ~ c2^16 * e^(16x) with
      ~1e-6 exp-weighted bias for x*16 ~ N(0,1); host divides partials
      by c2^16. All arithmetic stays in the DVE's f32 pipeline.
  - Host computes BCE + truncated KL in f64 exactly as the reference
    does on the selected support.
"""

import sys

import numpy as np

sys.path.insert(0, "/opt/trn_rl_repo")

import concourse.bacc as bacc  # noqa: E402
import concourse.bass as bass  # noqa: E402,F401
import concourse.dve_ops as dops  # noqa: E402
import concourse.mybir as mybir  # noqa: E402
from concourse.bass_utils import run_bass_kernel_spmd  # noqa: E402
from concourse.dve_spec import (  # noqa: E402
    C0, C1, AluOp, Bin, Spec, Src0, lower,
)
from concourse.dve_uop import DveOpSpec  # noqa: E402
from concourse.tile import TileContext  # noqa: E402

F32 = mybir.dt.float32
FP16 = mybir.dt.float16
BF16 = mybir.dt.bfloat16
ALU = mybir.AluOpType
ACTF = mybir.ActivationFunctionType
AX = mybir.AxisListType

B, L, V = 2, 2048, 32000
N = B * L                  # 4096 rows
NCORES = 8
ROWS = N // NCORES         # 512 rows per core
P = 128                    # rows per tile (partition dim)
NT = ROWS // P             # 4 tiles per core
K = 32
SC = 8000                  # standard chunk width (packed f32 elements)
BLK = 4000                 # selection block width (top-8 per block)
NCAND = 64                 # 8 blocks * 8 candidates per tile
CA = 6600                  # columns handled by ACT exp per 8000-chunk
DW = SC - CA               # columns handled by DVE exp per 8000-chunk


def _plan(it):
    """Per-tile chunk plan: list of (start, width, act_width).

    Tile 0 leads with four 4000-col chunks so the first compute starts
    after a 2 MB DMA instead of 4 MB and the pipeline fills while the
    first full-size chunks stream in (shorter ramp). The very last
    chunk shifts ~1500 exp columns from ACT to DVE: the selection work
    (max8) all lands on DVE mid-kernel, so at the tail DVE would
    otherwise sit idle while ACT finishes."""
    ws = [4000] * 4 + [8000] * 2 if it == 0 else [8000] * 4
    out = []
    c0 = 0
    for i, w in enumerate(ws):
        ca = (w * CA) // 8000
        if it == NT - 1 and i == len(ws) - 1:
            ca = 5100
        out.append((c0, w, ca))
        c0 += w
    return out


_OCOLS_T = [4 * len(_plan(it)) for it in range(NT)]   # accum cols per tile
_OOFF = [sum(_OCOLS_T[:i]) for i in range(NT)]        # per-tile col offsets
XSCALE = 16.0              # host pre-divides logits by this (exact in fp16)
EC1 = 1.943617330549676    # fused-exp fit: q = (x + EC1)*x + EC2
EC2 = 1.9421392252956275
ESCALE = EC2 ** 16         # DVE partials are scaled by this


def _register_exp_ops():
    """Monkeypatch the fused exp custom DVE op into the concourse
    registry. Pure-python registration: the per-NEFF ucode table is
    generated from the Spec at compile time (dve_table_for_ops), so no
    repo files change. The sha is computed at runtime to satisfy the
    drift check."""
    if any(op.name == "EXP16_SUM_ANT" for op in dops.OPS):
        return

    q = (Src0 + C0) * Src0 + C1
    for _ in range(4):
        q = Bin(AluOp.MULTIPLY, q, q)

    def ref(in0, in1, c0, c1, c2):
        x = in0.astype(np.float32)
        qq = (x + np.float32(c0)) * x + np.float32(c1)
        for _ in range(4):
            qq = qq * qq
        return qq, qq.sum(axis=-1, keepdims=True).astype(np.float32)

    sp = Spec(body=q, accum=AluOp.ADD, reference=ref)
    nm = "EXP16_SUM_ANT"
    dops._SUB_OPCODE_FOR_NAME[nm] = max(dops._SUB_OPCODE_FOR_NAME.values()) + 1
    assert dops._SUB_OPCODE_FOR_NAME[nm] < 0x20
    sha = DveOpSpec(name=nm, opcode=dops.get_dve_sub_opcode(nm),
                    uops=lower(sp, ver="v3"), rd1_en=False).sha("v3")
    op = dops.DveOp(nm, sp, subdim=False, uops_sha={"v3": sha})
    dops.OPS.append(op)
    dops.CUSTOM_DVE_SPECS[nm] = sp


_register_exp_ops()
_EXP16 = next(op for op in dops.OPS if op.name == "EXP16_SUM_ANT")


def build_nc(nt=NT):
    rows = nt * P
    nc = bacc.Bacc("TRN2", debug=False)
    ocols = sum(_OCOLS_T)
    p_in = nc.declare_dram_parameter("p", [rows, V], F32, isOutput=False)
    o_out = nc.declare_dram_parameter("o", [P, ocols], F32, isOutput=True)
    c_out = nc.declare_dram_parameter("c", [P, NCAND * nt], F32, isOutput=True)

    with TileContext(nc) as tc:
        with (
            tc.tile_pool(name="pck", bufs=4) as pck,
            tc.tile_pool(name="cnd", bufs=2) as cnd,
            tc.tile_pool(name="singles", bufs=1) as singles,
        ):
            max_ca = max(ca for it in range(nt) for _, _, ca in _plan(it))
            max_dw = max(w - ca for it in range(nt) for _, w, ca in _plan(it))
            out_t = singles.tile([P, ocols], F32)
            dump_a = singles.tile([P, max_ca], FP16)    # ACT exp dump
            dump_v = singles.tile([P, max_dw], BF16)    # DVE exp16 dump

            for it in range(nt):
                r0 = it * P
                oc = _OOFF[it]  # accum columns base (4 per chunk)
                cand = cnd.tile([P, NCAND], F32, tag="cand")
                g = 0  # block counter within the tile

                for u, (c0, w, ca) in enumerate(_plan(it)):
                    up = pck.tile([P, w], F32, tag=f"p{w}",
                                  name=f"p{it}_{u}")
                    nc.sync.dma_start(
                        out=up, in_=p_in[r0:r0 + P, c0:c0 + w])

                    # ACT: S_t / S_s partials over cols [0, ca): strided
                    # fp16 exp(16*x) with free accumulator outputs.
                    af = up[:, 0:ca].bitcast(FP16)
                    nc.scalar.activation(
                        out=dump_a[:, 0:ca], in_=af[:, 1::2], func=ACTF.Exp,
                        scale=XSCALE,
                        accum_out=out_t[:, oc:oc + 1],
                    )
                    nc.scalar.activation(
                        out=dump_a[:, 0:ca], in_=af[:, 0::2], func=ACTF.Exp,
                        scale=XSCALE,
                        accum_out=out_t[:, oc + 1:oc + 2],
                    )

                    # DVE: c2^16 * exp partials over cols [ca, w)
                    dv = up[:, ca:w].bitcast(FP16)
                    nc.vector._custom_dve(
                        _EXP16, out=dump_v[:, 0:w - ca], in0=dv[:, 1::2],
                        s0=EC1, s1=EC2,
                        accum_out=out_t[:, oc + 2:oc + 3],
                    )
                    nc.vector._custom_dve(
                        _EXP16, out=dump_v[:, 0:w - ca], in0=dv[:, 0::2],
                        s0=EC1, s1=EC2,
                        accum_out=out_t[:, oc + 3:oc + 4],
                    )
                    oc += 4

                    # per-block top-8 of packed pairs -> candidate tile
                    for b0 in range(0, w, BLK):
                        nc.vector.max(
                            out=cand[:, g * 8:(g + 1) * 8],
                            in_=up[:, b0:b0 + BLK],
                        )
                        g += 1

                assert g * 8 == NCAND
                nc.sync.dma_start(
                    out=c_out[:, it * NCAND:(it + 1) * NCAND], in_=cand)

            nc.sync.dma_start(out=o_out[:, :], in_=out_t[:, :])

    nc.finalize()
    return nc


_NC_CACHE = None


def _get_nc():
    global _NC_CACHE
    if _NC_CACHE is None:
        _NC_CACHE = build_nc()
    return _NC_CACHE


def pack_pairs(t2d, s2d):
    """(N, V) f32 teacher/student -> packed u32
    (fp16(t/16)<<16 | fp16(s/16)) viewed as f32."""
    th = (t2d / XSCALE).astype(np.float16).view(np.uint16).astype(np.uint32)
    sh = (s2d / XSCALE).astype(np.float16).view(np.uint16).astype(np.uint32)
    return ((th << 16) | sh).view(np.float32)


def run_device(t2d, s2d, trace=False, **kw):
    """t2d/s2d: (N, V) float32. Returns BassKernelResults."""
    nc = _get_nc()
    p2d = pack_pairs(t2d, s2d)
    in_maps = []
    for c in range(NCORES):
        sl = slice(c * ROWS, (c + 1) * ROWS)
        in_maps.append({"p": np.ascontiguousarray(p2d[sl])})
    return run_bass_kernel_spmd(nc, in_maps, list(range(NCORES)), trace=trace,
                                **kw)


def _gather(res):
    """Device results -> (cand, s_t, s_s)."""
    cand = np.empty((N, NCAND), dtype=np.float32)
    s_t = np.empty(N, dtype=np.float64)
    s_s = np.empty(N, dtype=np.float64)
    for c in range(NCORES):
        o = np.asarray(res.results[c]["o"])  # [P, OCOLS*NT] f32
        cd = np.asarray(res.results[c]["c"])  # [P, NCAND*NT] f32
        for it in range(NT):
            r = slice(c * ROWS + it * P, c * ROWS + (it + 1) * P)
            cand[r] = cd[:, it * NCAND:(it + 1) * NCAND]
            o64 = o[:, _OOFF[it]:_OOFF[it] + _OCOLS_T[it]].astype(np.float64)
            # per chunk: st_a | ss_a | st_d | ss_d
            s_t[r] = o64[:, 0::4].sum(1) + o64[:, 2::4].sum(1) / ESCALE
            s_s[r] = o64[:, 1::4].sum(1) + o64[:, 3::4].sum(1) / ESCALE
    return cand, s_t, s_s


def kernel(student_logits, teacher_logits):
    s2d = np.asarray(student_logits, dtype=np.float32).reshape(N, V)
    t2d = np.asarray(teacher_logits, dtype=np.float32).reshape(N, V)
    res = run_device(t2d, s2d)
    cand, s_t, s_s = _gather(res)

    # sanity net: a (rare, first-execution) corrupted run shows up as
    # non-finite or non-positive sums / out-of-range teacher values.
    tmax = (cand.view(np.uint32)[:, 0] >> 16).astype(np.uint16).view(
        np.float16).astype(np.float64)
    if (not np.isfinite(s_t).all() or not np.isfinite(s_s).all()
            or (s_t <= 0).any() or (s_s <= 0).any()
            or not np.isfinite(tmax).all()):
        res = run_device(t2d, s2d)
        cand, s_t, s_s = _gather(res)

    # host top-32 of the candidate pairs per row (same f32 ordering as
    # the device max8: packed high bits = fp16 teacher)
    top32 = -np.sort(-cand, axis=1)[:, :K]
    p32 = top32.view(np.uint32)

    # unpack fp16 halves (values are x/16) -> t32, s32 (f64)
    t32 = (p32 >> 16).astype(np.uint16).view(np.float16).astype(
        np.float64) * XSCALE
    s32 = (p32 & 0xFFFF).astype(np.uint16).view(np.float16).astype(
        np.float64) * XSCALE

    # host finals in f64, replicating the reference on this support
    a_t = np.exp(t32).sum(1)
    p_t = a_t / s_t
    p_s = np.exp(s32).sum(1) / s_s

    log_ps = np.maximum(np.log(p_s), -100.0)
    log_1mps = np.maximum(np.log1p(-p_s), -100.0)
    loss_b = np.mean(-(p_t * log_ps + (1.0 - p_t) * log_1mps))

    th = t32 / 2.0
    sh = s32 / 2.0
    log_p = th - (np.log(np.exp(th - th.max(1, keepdims=True)).sum(1))
                  + th.max(1)).reshape(-1, 1)
    log_q = sh - (np.log(np.exp(sh - sh.max(1, keepdims=True)).sum(1))
                  + sh.max(1)).reshape(-1, 1)
    p = np.exp(log_p)
    loss_t = (p * (log_p - log_q)).sum(1).mean()

    return np.float32(loss_b + p_t.mean() * 4.0 * loss_t)
